# revision 4
# baseline (speedup 1.0000x reference)
"""Trainium2 Bass kernel for GQA attention block (B=2, S=2048, H=2048,
16 q-heads / 4 kv-heads, head_dim=128, RoPE, causal) on 8 NeuronCores.

Sharding: core c -> batch b = c // 4, kv-group g = c % 4
  (q heads 4g..4g+3, kv head g).  Each core computes its batch's
  attention for its 4 query heads plus the partial output projection
  over its 512 hidden columns of w_o; host sums the 4 partials per batch.

On-chip layouts (per core):
  qT/kT  [head_dim=128 part, S free]   (projection emits transposed)
  v      [S part-blocks,   head_dim]   (for PV matmul lhsT)
  scoresT[sk part, sq free]  -> exp -> PV accumulates out^T [d, sq]
  softmax denominator via ones-matmul (partition reduce on PE)
  o-proj emits out[s, o] directly (no host transpose needed)

All heavy matmuls run in float32r (full PE speed; ~1e-4 rel err).
"""

import math
import numpy as np

import concourse.bacc as bacc
import concourse.mybir as mybir
import concourse.tile as tile
from concourse.bass_utils import run_bass_kernel_spmd

F32 = mybir.dt.float32
F32R = mybir.dt.float32r
AF = mybir.ActivationFunctionType

S = 2048
H = 2048
D = 128            # head dim
KT = 16            # contraction tiles over hidden (2048/128)
HALF = 1024        # s-half width for the projection phase
NQ = 512           # query block width in attention
NUM_Q_LOCAL = 4    # q heads per core
SCALE = 1.0 / math.sqrt(D)
NEG = -1.0e9

_CACHED = {}


def build_nc(mm_dt=F32R):
    nc = bacc.Bacc(None, target_bir_lowering=False)
    hT = nc.dram_tensor("hT", [H, S], mm_dt, kind="ExternalInput")
    wqk = nc.dram_tensor("wqk", [KT, 5, 128, 128], mm_dt, kind="ExternalInput")
    wv = nc.dram_tensor("wv", [KT, 128, 128], mm_dt, kind="ExternalInput")
    cosT = nc.dram_tensor("cosT", [D, S], F32, kind="ExternalInput")
    sinT = nc.dram_tensor("sinT", [D, S], F32, kind="ExternalInput")
    masks = nc.dram_tensor("masks", [128, 4 * NQ], F32, kind="ExternalInput")
    wo = nc.dram_tensor("wo", [4, 128, H], mm_dt, kind="ExternalInput")
    out = nc.dram_tensor("out", [S, H], F32, kind="ExternalOutput")

    with tile.TileContext(nc) as tc:
        with tc.tile_pool(name="persist", bufs=1) as pp:
            # ---- persistent tiles (live across phases) ----
            qk = [pp.tile([128, S], F32, name=f"qk{i}", tag=f"qk{i}") for i in range(5)]
            v_sb = pp.tile([128, S], mm_dt, tag="v")
            cos_sb = pp.tile([128, S], F32, tag="cos")
            sin_sb = pp.tile([128, S], F32, tag="sin")
            masks_sb = pp.tile([128, 4 * NQ], F32, tag="masks")
            ones_f = pp.tile([128, 128], F32, tag="onesf")
            ones_r = pp.tile([128, 1], mm_dt, tag="onesr")

            nc.sync.dma_start(out=cos_sb[:], in_=cosT[:])
            nc.sync.dma_start(out=sin_sb[:], in_=sinT[:])
            nc.sync.dma_start(out=masks_sb[:], in_=masks[:])
            nc.vector.memset(ones_f[:], 1.0)
            nc.vector.tensor_copy(ones_r[:], ones_f[:, 0:1])

            # ---- Phase 1: fused QKV projection (two s-halves) ----
            with (
                tc.tile_pool(name="ht", bufs=1) as htp,
                tc.tile_pool(name="wq", bufs=4) as wqp,
                tc.tile_pool(name="wvp", bufs=1) as wvp,
                tc.tile_pool(name="psq", bufs=2, space="PSUM") as psq,
                tc.tile_pool(name="psv", bufs=2, space="PSUM") as psv,
            ):
                wv_sb = wvp.tile([128, KT * 128], mm_dt, tag="wvw")
                for kt in range(KT):
                    nc.sync.dma_start(
                        out=wv_sb[:, kt * 128 : (kt + 1) * 128], in_=wv[kt]
                    )
                for half in range(2):
                    s0 = half * HALF
                    ht = [
                        htp.tile([128, HALF], mm_dt, name=f"ht{kt}", tag=f"ht{kt}")
                        for kt in range(KT)
                    ]
                    for kt in range(KT):
                        nc.sync.dma_start(
                            out=ht[kt][:],
                            in_=hT[kt * 128 : (kt + 1) * 128, s0 : s0 + HALF],
                        )
                    # q (4 heads) + k: output transposed [d, s]
                    for i in range(5):
                        ps = psq.tile([128, HALF], F32, tag="psq")
                        for kt in range(KT):
                            w = wqp.tile([128, 128], mm_dt, tag="wq")
                            nc.sync.dma_start(out=w[:], in_=wqk[kt, i])
                            for n in range(HALF // NQ):
                                nc.tensor.matmul(
                                    ps[:, n * NQ : (n + 1) * NQ],
                                    lhsT=w[:],
                                    rhs=ht[kt][:, n * NQ : (n + 1) * NQ],
                                    start=(kt == 0),
                                    stop=(kt == KT - 1),
                                )
                        nc.scalar.copy(qk[i][:, s0 : s0 + HALF].bitcast(mm_dt), ps[:])
                    # v: output [s, d] layout, s-blocks on partitions
                    for sb in range(HALF // 128):
                        psvt = psv.tile([128, 128], F32, tag="psv")
                        for kt in range(KT):
                            nc.tensor.matmul(
                                psvt[:],
                                lhsT=ht[kt][:, sb * 128 : (sb + 1) * 128],
                                rhs=wv_sb[:, kt * 128 : (kt + 1) * 128],
                                start=(kt == 0),
                                stop=(kt == KT - 1),
                            )
                        nc.vector.tensor_copy(
                            v_sb[:, s0 + sb * 128 : s0 + (sb + 1) * 128], psvt[:]
                        )

            # ---- Phase 1.5: RoPE on q heads and k ----
            with tc.tile_pool(name="rope", bufs=2) as rp:
                for i in range(5):
                    rot = rp.tile([128, S], F32, tag="rot")
                    tmp = rp.tile([128, S], F32, tag="tmp")
                    nc.sync.dma_start(out=rot[0:64, :], in_=qk[i][64:128, :])
                    nc.sync.dma_start(out=rot[64:128, :], in_=qk[i][0:64, :])
                    nc.vector.tensor_mul(tmp[:], rot[:], sin_sb[:])
                    nc.vector.tensor_mul(rot[:], qk[i][:], cos_sb[:])
                    nc.vector.tensor_add(qk[i][:].bitcast(mm_dt), rot[:], tmp[:])

            # ---- Phase 2 + 3 interleaved per query block ----
            with (
                tc.tile_pool(name="attn", bufs=1) as ap,
                tc.tile_pool(name="wop", bufs=1) as wop,
                tc.tile_pool(name="epool", bufs=4) as ep,
                tc.tile_pool(name="small", bufs=4) as sp,
                tc.tile_pool(name="outp", bufs=3) as op_,
                tc.tile_pool(name="pss", bufs=2, space="PSUM") as pss,
                tc.tile_pool(name="pspv", bufs=2, space="PSUM") as pspv,
                tc.tile_pool(name="psden", bufs=1, space="PSUM") as psden,
                tc.tile_pool(name="psbc", bufs=1, space="PSUM") as psbc,
                tc.tile_pool(name="pso", bufs=2, space="PSUM") as pso,
            ):
                attnT = [
                    ap.tile([128, S], mm_dt, name=f"at{h}", tag=f"at{h}")
                    for h in range(4)
                ]
                wo_sb = [
                    wop.tile([128, H], mm_dt, name=f"wo{kb}", tag=f"wo{kb}")
                    for kb in range(4)
                ]
                for kb in range(4):
                    nc.sync.dma_start(out=wo_sb[kb][:], in_=wo[kb])

                kT = qk[4][:].bitcast(mm_dt)
                for qb in range(S // NQ):
                    q0 = qb * NQ
                    nj = 4 * qb + 4
                    for h in range(NUM_Q_LOCAL):
                        qT = qk[h][:].bitcast(mm_dt)
                        pv = pspv.tile([128, NQ], F32, tag="pv")
                        den = psden.tile([1, NQ], F32, tag="den")
                        for j in range(nj):
                            sps = pss.tile([128, NQ], F32, tag="sc")
                            nc.tensor.matmul(
                                sps[:],
                                lhsT=kT[:, j * 128 : (j + 1) * 128],
                                rhs=qT[:, q0 : q0 + NQ],
                                start=True,
                                stop=True,
                            )
                            r4 = j - 4 * qb
                            if r4 >= 0:
                                nc.vector.tensor_add(
                                    sps[:],
                                    sps[:],
                                    masks_sb[:, r4 * NQ : (r4 + 1) * NQ],
                                )
                            e = ep.tile([128, NQ], mm_dt, tag="e")
                            nc.scalar.activation(e[:], sps[:], AF.Exp, scale=SCALE)
                            nc.tensor.matmul(
                                pv[:],
                                lhsT=v_sb[:, j * 128 : (j + 1) * 128],
                                rhs=e[:],
                                start=(j == 0),
                                stop=(j == nj - 1),
                            )
                            nc.tensor.matmul(
                                den[:],
                                lhsT=ones_r[:],
                                rhs=e[:],
                                start=(j == 0),
                                stop=(j == nj - 1),
                            )
                        rec = sp.tile([1, NQ], F32, tag="rec")
                        nc.vector.reciprocal(rec[:], den[:])
                        bc = psbc.tile([128, NQ], F32, tag="bc")
                        nc.tensor.matmul(
                            bc[:],
                            lhsT=ones_f[0:1, :],
                            rhs=rec[:],
                            start=True,
                            stop=True,
                        )
                        bcs = sp.tile([128, NQ], F32, tag="bcs")
                        nc.scalar.copy(bcs[:], bc[:])
                        nc.vector.tensor_mul(
                            attnT[h][:, q0 : q0 + NQ], pv[:], bcs[:]
                        )
                    # o-projection for the 4 s-blocks of this query block
                    for sbl in range(NQ // 128):
                        sb = qb * 4 + sbl
                        for n in range(H // NQ):
                            pst = pso.tile([128, NQ], F32, tag="po")
                            for kb in range(4):
                                nc.tensor.matmul(
                                    pst[:],
                                    lhsT=attnT[kb][:, sb * 128 : (sb + 1) * 128],
                                    rhs=wo_sb[kb][:, n * NQ : (n + 1) * NQ],
                                    start=(kb == 0),
                                    stop=(kb == 3),
                                )
                            osb = op_.tile([128, NQ], F32, tag="osb")
                            nc.scalar.copy(osb[:], pst[:])
                            nc.sync.dma_start(
                                out=out[
                                    sb * 128 : (sb + 1) * 128, n * NQ : (n + 1) * NQ
                                ],
                                in_=osb[:],
                            )

    nc.compile()
    return nc


def _prep_inputs(hidden_states, cos, sin, w_qkv, w_o):
    """Build the 8 per-core input maps (host-side shard + transpose)."""
    hidden_states = np.asarray(hidden_states, dtype=np.float32)
    cos = np.asarray(cos, dtype=np.float32)
    sin = np.asarray(sin, dtype=np.float32)
    w_qkv = np.asarray(w_qkv, dtype=np.float32)
    w_o = np.asarray(w_o, dtype=np.float32)

    cosT = np.ascontiguousarray(cos.T)
    sinT = np.ascontiguousarray(sin.T).copy()
    sinT[0:64] *= -1.0  # rotate_half sign folded into sin

    # masks[sk, r4*512 + sq] = 0 if (r4*128 + sk) <= sq else NEG
    sk = np.arange(128)[:, None]
    sq = np.arange(NQ)[None, :]
    masks = np.concatenate(
        [np.where(r4 * 128 + sk <= sq, 0.0, NEG) for r4 in range(4)], axis=1
    ).astype(np.float32)

    hT = [np.ascontiguousarray(hidden_states[b].T) for b in range(2)]

    in_maps = []
    for c in range(8):
        b, g = divmod(c, 4)
        W5 = np.stack(
            [w_qkv[(4 * g + i) * 128 : (4 * g + i + 1) * 128] for i in range(4)]
            + [w_qkv[(16 + g) * 128 : (17 + g) * 128]]
        )  # [5, 128 m, 2048 h]
        wqk_pack = np.ascontiguousarray(
            W5.transpose(2, 0, 1).reshape(KT, 128, 5, 128).transpose(0, 2, 1, 3)
        )  # [kt, i, p(h), m]
        v_rows = w_qkv[(20 + g) * 128 : (21 + g) * 128]  # [128 d, 2048 h]
        wv_pack = np.ascontiguousarray(v_rows.T.reshape(KT, 128, 128))
        wo_pack = np.ascontiguousarray(
            np.stack(
                [
                    w_o[:, (4 * g + kb) * 128 : (4 * g + kb + 1) * 128].T
                    for kb in range(4)
                ]
            )
        )  # [4, 128 hd, 2048 o]
        in_maps.append(
            dict(
                hT=hT[b],
                wqk=wqk_pack,
                wv=wv_pack,
                cosT=cosT,
                sinT=sinT,
                masks=masks,
                wo=wo_pack,
            )
        )
    return in_maps


def run(hidden_states, cos, sin, w_qkv, w_o, trace=False, **trace_kwargs):
    if "nc" not in _CACHED:
        _CACHED["nc"] = build_nc()
    nc = _CACHED["nc"]
    in_maps = _prep_inputs(hidden_states, cos, sin, w_qkv, w_o)
    res = run_bass_kernel_spmd(
        nc, in_maps, core_ids=list(range(8)), trace=trace, **trace_kwargs
    )
    outs = [res.results[c]["out"] for c in range(8)]
    full = np.stack(
        [
            outs[0] + outs[1] + outs[2] + outs[3],
            outs[4] + outs[5] + outs[6] + outs[7],
        ]
    ).astype(np.float32)
    return full, res


def kernel(hidden_states, cos, sin, w_qkv, w_o):
    full, _ = run(hidden_states, cos, sin, w_qkv, w_o, trace=False)
    return full


# revision 17
# speedup vs baseline: 276.8968x; 276.8968x over previous
"""Trainium2 Bass kernel for GQA attention block (B=2, S=2048, H=2048,
16 q-heads / 4 kv-heads, head_dim=128, RoPE, causal) on 8 NeuronCores.

Sharding: core c -> batch b = c // 4, kv-group g = c % 4
  (q heads 4g..4g+3, kv head g).  Each core computes its batch's
  attention for its 4 query heads plus the partial output projection
  over its 512 hidden columns of w_o; host sums the 4 partials per batch.

On-chip layouts (per core):
  qT/kT/vT [head_dim=128 part, S free]  (projection emits transposed)
  v        [S part-blocks, head_dim]    (PE transpose of vT; PV lhsT)
  scoresT  [sk part, sq free] -> exp -> PV accumulates out^T [d, sq]
  softmax denominator via ones-matmul (partition reduce on PE),
  reciprocal broadcast via GpSimd partition_broadcast
  o-proj emits out[s, o] directly (no host transpose needed)

All heavy matmuls run in float32r (full PE speed at N>=256; ~1e-4 rel err).
"""

import contextlib
import math
import numpy as np

import concourse.bacc as bacc
import concourse.mybir as mybir
import concourse.tile as tile
from concourse.bass_utils import run_bass_kernel_spmd
from concourse.masks import make_identity

F32 = mybir.dt.float32
F32R = mybir.dt.float32r
AF = mybir.ActivationFunctionType

S = 2048
H = 2048
D = 128            # head dim
KT = 16            # contraction tiles over hidden (2048/128)
HALF = 1024        # s-half width for the projection phase
NQ = 512           # query block width in attention
NUM_Q_LOCAL = 4    # q heads per core
SCALE = 1.0 / math.sqrt(D)
NEG = -1.0e9

_CACHED = {}


class _SkipBlockExc(Exception):
    pass


class _SkipBlock:
    """Context manager that skips its with-body entirely."""

    def __enter__(self):
        import sys
        self._tr = sys.gettrace()
        sys.settrace(lambda *a, **k: None)
        import inspect
        frame = inspect.currentframe().f_back
        frame.f_trace = self._trace
        return self

    def _trace(self, frame, event, arg):
        raise _SkipBlockExc

    def __exit__(self, exc_type, exc, tb):
        import sys
        sys.settrace(self._tr)
        return exc_type is _SkipBlockExc



def build_nc(mm_dt=F32R, loop_n=None, phases=(1, 2, 3)):
    nc = bacc.Bacc(None, target_bir_lowering=False)
    hT = nc.dram_tensor("hT", [H, S], mm_dt, kind="ExternalInput")
    wqk = nc.dram_tensor("wqk", [KT, 6, 128, 128], mm_dt, kind="ExternalInput")
    cosT = nc.dram_tensor("cosT", [D, S], F32, kind="ExternalInput")
    sinT = nc.dram_tensor("sinT", [D, S], F32, kind="ExternalInput")
    masks = nc.dram_tensor("masks", [128, 4 * NQ], F32, kind="ExternalInput")
    wo = nc.dram_tensor("wo", [4, 128, H], mm_dt, kind="ExternalInput")
    out = nc.dram_tensor("out", [S, H], F32, kind="ExternalOutput")

    with tile.TileContext(nc) as tc:
        with tc.tile_pool(name="persist", bufs=1) as pp:
          with (tc.For_i(0, loop_n, 1) if loop_n else contextlib.nullcontext()):
            # ---- persistent tiles (live across phases) ----
            qk = [pp.tile([128, S], F32, name=f"qk{i}", tag=f"qk{i}") for i in range(5)]
            v_sb = pp.tile([128, S], mm_dt, tag="v")
            cos_sb = pp.tile([128, S], F32, tag="cos")
            sin_sb = pp.tile([128, S], F32, tag="sin")
            ones_r = pp.tile([128, 1], mm_dt, tag="onesr")
            ident = pp.tile([128, 128], F32, tag="ident")

            # ---- Phase 1: fused QKV projection (s-quarters, resident weights)
            # + RoPE and v-transpose interleaved per quarter ----
            QW = 512
            NQT = S // QW
            with (
                tc.tile_pool(name="ht", bufs=2) as htp,
                tc.tile_pool(name="wq", bufs=1) as wqp,
                tc.tile_pool(name="vtp", bufs=1) as vtp,
                tc.tile_pool(name="rope", bufs=2) as rp,
                tc.tile_pool(name="psq", bufs=3, space="PSUM") as psq,
                tc.tile_pool(name="psv", bufs=2, space="PSUM") as psv,
            ):
                vT_sb = vtp.tile([128, S], F32, tag="vT")
                w_sb = [
                    wqp.tile([128, KT * 128], mm_dt, name=f"w{i}", tag=f"w{i}")
                    for i in range(6)
                ]
                for q in range(NQT):
                    s0 = q * QW
                    ht = [
                        htp.tile([128, QW], mm_dt, name=f"ht{kt}", tag=f"ht{kt}")
                        for kt in range(KT)
                    ]
                    # DMA in consumption order: per kt, the ht tile (+ weights on q==0)
                    for kt in range(KT):
                        nc.sync.dma_start(
                            out=ht[kt][:],
                            in_=hT[kt * 128 : (kt + 1) * 128, s0 : s0 + QW],
                        )
                        if q == 0:
                            for i in range(6):
                                nc.sync.dma_start(
                                    out=w_sb[i][:, kt * 128 : (kt + 1) * 128],
                                    in_=wqk[kt, i],
                                )
                    if q == 0:
                        nc.sync.dma_start(out=cos_sb[:], in_=cosT[:])
                        nc.sync.dma_start(out=sin_sb[:], in_=sinT[:])
                        nc.vector.memset(ones_r.bitcast(F32)[:], 1.0)
                        make_identity(nc, ident[:])
                    for i in range(6):
                        ps = psq.tile([128, QW], F32, tag="psq")
                        for kt in range(KT):
                            nc.tensor.matmul(
                                ps[:],
                                lhsT=w_sb[i][:, kt * 128 : (kt + 1) * 128],
                                rhs=ht[kt][:],
                                start=(kt == 0),
                                stop=(kt == KT - 1),
                            )
                        if i < 5:
                            # copy raw head quarter, then RoPE it in place
                            nc.vector.tensor_copy(qk[i][:, s0 : s0 + QW].bitcast(mm_dt), ps[:])
                            rot = rp.tile([128, QW], F32, tag="rot")
                            tmp = rp.tile([128, QW], F32, tag="tmp")
                            nc.sync.dma_start(
                                out=rot[0:64, :], in_=qk[i][64:128, s0 : s0 + QW]
                            )
                            nc.sync.dma_start(
                                out=rot[64:128, :], in_=qk[i][0:64, s0 : s0 + QW]
                            )
                            nc.gpsimd.tensor_mul(
                                tmp[:], rot[:], sin_sb[:, s0 : s0 + QW]
                            )
                            nc.vector.tensor_mul(
                                rot[:], qk[i][:, s0 : s0 + QW], cos_sb[:, s0 : s0 + QW]
                            )
                            nc.vector.tensor_add(
                                qk[i][:, s0 : s0 + QW].bitcast(mm_dt), rot[:], tmp[:]
                            )
                        else:
                            nc.vector.tensor_copy(vT_sb[:, s0 : s0 + QW], ps[:])
                            for sbl in range(QW // 128):
                                sb = q * (QW // 128) + sbl
                                psvt = psv.tile([128, 128], F32, tag="psv")
                                nc.tensor.transpose(
                                    psvt[:],
                                    vT_sb[:, sb * 128 : (sb + 1) * 128],
                                    ident[:],
                                )
                                nc.vector.tensor_copy(
                                    v_sb[:, sb * 128 : (sb + 1) * 128], psvt[:]
                                )

            # ---- Phase 2 + 3 interleaved per query block ----
            with (
                contextlib.nullcontext() if 2 in phases else _SkipBlock(),
                tc.tile_pool(name="attn", bufs=1) as ap,
                tc.tile_pool(name="wop", bufs=1) as wop,
                tc.tile_pool(name="epool", bufs=6) as ep,
                tc.tile_pool(name="small", bufs=4) as sp,
                tc.tile_pool(name="pss", bufs=3, space="PSUM") as pss,
                tc.tile_pool(name="pspv", bufs=2, space="PSUM") as pspv,
                tc.tile_pool(name="psden", bufs=1, space="PSUM") as psden,
                tc.tile_pool(name="pso", bufs=2, space="PSUM") as pso,
            ):
                attnT = [
                    ap.tile([128, S], mm_dt, name=f"at{h}", tag=f"at{h}")
                    for h in range(4)
                ]
                wo_sb = [
                    wop.tile([128, H], mm_dt, name=f"wo{kb}", tag=f"wo{kb}")
                    for kb in range(4)
                ]
                masks_sb = ap.tile([128, 4 * NQ], F32, tag="masks")
                nc.sync.dma_start(out=masks_sb[:], in_=masks[:])

                kT = qk[4][:].bitcast(mm_dt)
                for qb in range(S // NQ):
                    q0 = qb * NQ
                    nj = 4 * qb + 4
                    for h in range(NUM_Q_LOCAL):
                        qT = qk[h][:].bitcast(mm_dt)
                        pv = pspv.tile([128, NQ], F32, tag="pv")
                        den = psden.tile([1, NQ], F32, tag="den")
                        for j in range(nj):
                            r4 = j - 4 * qb
                            # diagonal blocks: columns sq < r4*128 are fully
                            # masked -> narrow the whole j-chain to [off:NQ)
                            off = max(0, r4) * 128
                            sps = pss.tile([128, NQ], F32, tag="sc")
                            nc.tensor.matmul(
                                sps[:, off:NQ],
                                lhsT=kT[:, j * 128 : (j + 1) * 128],
                                rhs=qT[:, q0 + off : q0 + NQ],
                                start=True,
                                stop=True,
                            )
                            if r4 >= 0:
                                nc.vector.tensor_add(
                                    sps[:, off:NQ],
                                    sps[:, off:NQ],
                                    masks_sb[:, r4 * NQ + off : (r4 + 1) * NQ],
                                )
                            e = ep.tile([128, NQ], mm_dt, tag="e")
                            nc.scalar.activation(
                                e[:, off:NQ], sps[:, off:NQ], AF.Exp, scale=SCALE
                            )
                            nc.tensor.matmul(
                                pv[:, off:NQ],
                                lhsT=v_sb[:, j * 128 : (j + 1) * 128],
                                rhs=e[:, off:NQ],
                                start=(j == 0),
                                stop=(j == nj - 1),
                            )
                            nc.tensor.matmul(
                                den[:, off:NQ],
                                lhsT=ones_r[:],
                                rhs=e[:, off:NQ],
                                start=(j == 0),
                                stop=(j == nj - 1),
                            )
                        rec = sp.tile([1, NQ], F32, tag="rec")
                        nc.vector.reciprocal(rec[:], den[:])
                        bcs = sp.tile([128, NQ], F32, tag="bcs")
                        nc.gpsimd.partition_broadcast(bcs[:], rec[:])
                        nc.vector.tensor_mul(
                            attnT[h][:, q0 : q0 + NQ], pv[:], bcs[:]
                        )
                    if qb == 0 and 3 in phases:
                        for kb in range(4):
                            for wh in range(2):
                                nc.sync.dma_start(
                                    out=wo_sb[kb][:, wh * 1024 : (wh + 1) * 1024],
                                    in_=wo[kb][:, wh * 1024 : (wh + 1) * 1024],
                                )
                    # o-projection for the 4 s-blocks of this query block
                    for sbl in range(NQ // 128 if 3 in phases else 0):
                        sb = qb * 4 + sbl
                        for n in range(H // NQ):
                            pst = pso.tile([128, NQ], F32, tag="po")
                            for kb in range(4):
                                nc.tensor.matmul(
                                    pst[:],
                                    lhsT=attnT[kb][:, sb * 128 : (sb + 1) * 128],
                                    rhs=wo_sb[kb][:, n * NQ : (n + 1) * NQ],
                                    start=(kb == 0),
                                    stop=(kb == 3),
                                )
                            osb = sp.tile([128, NQ], F32, tag="osb")
                            nc.vector.tensor_copy(osb[:], pst[:])
                            nc.sync.dma_start(
                                out=out[
                                    sb * 128 : (sb + 1) * 128, n * NQ : (n + 1) * NQ
                                ],
                                in_=osb[:],
                            )

    nc.compile()
    return nc


def _prep_inputs(hidden_states, cos, sin, w_qkv, w_o):
    """Build the 8 per-core input maps (host-side shard + transpose)."""
    hidden_states = np.asarray(hidden_states, dtype=np.float32)
    cos = np.asarray(cos, dtype=np.float32)
    sin = np.asarray(sin, dtype=np.float32)
    w_qkv = np.asarray(w_qkv, dtype=np.float32)
    w_o = np.asarray(w_o, dtype=np.float32)

    cosT = np.ascontiguousarray(cos.T)
    sinT = np.ascontiguousarray(sin.T).copy()
    sinT[0:64] *= -1.0  # rotate_half sign folded into sin

    # masks[sk, r4*512 + sq] = 0 if (r4*128 + sk) <= sq else NEG
    sk = np.arange(128)[:, None]
    sq = np.arange(NQ)[None, :]
    masks = np.concatenate(
        [np.where(r4 * 128 + sk <= sq, 0.0, NEG) for r4 in range(4)], axis=1
    ).astype(np.float32)

    hT = [np.ascontiguousarray(hidden_states[b].T) for b in range(2)]

    in_maps = []
    for c in range(8):
        b, g = divmod(c, 4)
        W6 = np.stack(
            [w_qkv[(4 * g + i) * 128 : (4 * g + i + 1) * 128] for i in range(4)]
            + [w_qkv[(16 + g) * 128 : (17 + g) * 128]]
            + [w_qkv[(20 + g) * 128 : (21 + g) * 128]]
        )  # [6, 128 m, 2048 h]
        wqk_pack = np.ascontiguousarray(
            W6.transpose(2, 0, 1).reshape(KT, 128, 6, 128).transpose(0, 2, 1, 3)
        )  # [kt, i, p(h), m]
        wo_pack = np.ascontiguousarray(
            np.stack(
                [
                    w_o[:, (4 * g + kb) * 128 : (4 * g + kb + 1) * 128].T
                    for kb in range(4)
                ]
            )
        )  # [4, 128 hd, 2048 o]
        in_maps.append(
            dict(
                hT=hT[b],
                wqk=wqk_pack,
                cosT=cosT,
                sinT=sinT,
                masks=masks,
                wo=wo_pack,
            )
        )
    return in_maps


def run(hidden_states, cos, sin, w_qkv, w_o, trace=False, **trace_kwargs):
    if "nc" not in _CACHED:
        _CACHED["nc"] = build_nc()
    nc = _CACHED["nc"]
    in_maps = _prep_inputs(hidden_states, cos, sin, w_qkv, w_o)
    res = run_bass_kernel_spmd(
        nc, in_maps, core_ids=list(range(8)), trace=trace, **trace_kwargs
    )
    outs = [res.results[c]["out"] for c in range(8)]
    full = np.stack(
        [
            outs[0] + outs[1] + outs[2] + outs[3],
            outs[4] + outs[5] + outs[6] + outs[7],
        ]
    ).astype(np.float32)
    return full, res


def kernel(hidden_states, cos, sin, w_qkv, w_o):
    full, _ = run(hidden_states, cos, sin, w_qkv, w_o, trace=False)
    return full
